# revision 9
# baseline (speedup 1.0000x reference)
"""2D DCT-II (4096x4096, fp32) on 8 TRN2 NeuronCores.

out = C0 @ x @ C1^T with C0 = C1 = C, C[k, i] = cos(pi*(2i+1)*k/(2N)).

Fast-DCT folding via the basis reflection symmetries
    C[u, N-1-i]   = (-1)^u     * C[u, i]        (level 1, both axes)
    C[v, N/2-1-j] = (-1)^(v/2) * C[v, j]  (v even; level 2, column axis)

level 1 (both stages, folded on the HOST -> half FLOPs + half HBM):
  - cores 0-3 own even output rows u, cores 4-7 odd rows;
  - host supplies doubly-folded x quarters xa/xb [2048,2048] (feeding
    even-v / odd-v outputs) and basis slices;
level 2 (column axis only, even v split into v%4==0 / v%4==2):
  - xa arrives with its columns permuted [0..1023, 2047..1024] so the
    stage-1 intermediate tiles pair reflection partners at identical
    partition offsets; a 16-op DVE butterfly (running under stage 1's
    remaining matmuls) then yields the quarter-folded T2E/T2O, and the
    v%4 sections contract over only 1024 elements.

Device pipeline per core (all matmuls fp32r = full-rate FP22):
  stage 1: T(E|O)^T[j', m] = sum_i' x(a|b)[i', j'] * c0tp[i', m]
     lhsT = x tile (streamed, 1 MB DMAs), rhs = c0tp (SBUF-resident)
     -> 512 matmuls; intermediates land transposed in SBUF, exactly the
     stationary layout stage 2 needs.
  butterfly: t2e (in-place over TE') / t2o = TE'[j''] -+ TE'[j''+1024]
  stage 2: v%4==0: sum_{j''<1024} t2e^T * C[4v'', j'']     ( 64 matmuls)
           v%4==2: sum_{j''<1024} t2o^T * C[4v''+2, j'']   ( 64 matmuls)
           v odd:  sum_{j'<2048}  TO^T  * C[2v'+1, j']     (256 matmuls)
     rhs = basis (streamed), lhsT = intermediates (SBUF-resident).
  Output leaves in section-packed columns [v0 | v2 | vodd]; the host
  de-interleaves (pure numpy slicing).

PSUM: 4-bank accumulation groups alternate between two bank sets so a
group's drain (DVE/ACT copies, alternating) overlaps the next group's
matmuls. Total per-core: 896 matmuls (~203 us PE) + ~60 MB HBM.
"""

import math

import numpy as np

import concourse.mybir as mybir
import concourse.tile as tile
from concourse import bacc
from concourse.bass_utils import run_bass_kernel_spmd

N = 4096
H = N // 2  # 2048: level-1 folded contraction
Q = N // 4  # 1024: level-2 folded contraction
P = 128
HT = H // P  # 16
QT = Q // P  # 8
NCORES = 8
RB = 512  # output rows per core
G = 512  # column-group / matmul moving width
KQ = 4  # k-tiles per streaming DMA (1 MB)

f32 = mybir.dt.float32
f32r = mybir.dt.float32r

_CACHE = {}


def _build():
    nc = bacc.Bacc("TRN2", target_bir_lowering=False, debug=False)
    xa_d = nc.dram_tensor("xa", [H, H], f32r, kind="ExternalInput")
    xb_d = nc.dram_tensor("xb", [H, H], f32r, kind="ExternalInput")
    c0tp_d = nc.dram_tensor("c0tp", [H, RB], f32r, kind="ExternalInput")
    c1v02_d = nc.dram_tensor("c1v02", [Q, H], f32r, kind="ExternalInput")
    c1vo_d = nc.dram_tensor("c1vo", [H, H], f32r, kind="ExternalInput")
    out_d = nc.dram_tensor("out", [RB, N], f32, kind="ExternalOutput")

    state = {"ggc": 0}

    with tile.TileContext(nc) as tc:
        with (
            tc.tile_pool(name="persist", bufs=1) as persist,
            tc.tile_pool(name="xin", bufs=4) as xin,
            tc.tile_pool(name="cin", bufs=5) as cin,
            tc.tile_pool(name="osb", bufs=3) as osb,
            tc.tile_pool(name="ps", bufs=1, space="PSUM") as ps,
        ):
            c0tp_sb = persist.tile([P, HT, RB], f32r, tag="c0", name="c0tp_sb")
            # TE' (permuted) / TO intermediates: [j', m] as [128, 16, 512]
            t_sb = [
                persist.tile([P, HT, RB], f32r, tag=f"t{h}", name=f"t{h}_sb")
                for h in range(2)
            ]
            # level-2 odd-sign butterfly output (t2e overwrites t0 in place)
            t2o_sb = persist.tile([P, QT, RB], f32r, tag="t2o", name="t2o_sb")

            def banks(n=4):
                g = state["ggc"]
                state["ggc"] += 1
                return [
                    ps.tile(
                        [P, G], f32, tag=f"ps{(g % 2) * 4 + i}",
                        name=f"ps{(g % 2) * 4 + i}",
                    )
                    for i in range(n)
                ]

            def drain(bk, mb, dst):
                # alternate DVE/ACT so section-end drains parallelize
                if mb % 2 == 0:
                    nc.vector.tensor_copy(dst, bk[:])
                else:
                    nc.scalar.copy(dst, bk[:])

            # PE warm-up: the HAM clock gate needs ~3.4 us of sustained
            # matmul activity to lift the PE from 1.2 to 2.4 GHz, and the
            # first real matmul can't start until ~0.5 MB of operands
            # land (~10 us incl. preamble). Chew zeros meanwhile so the
            # real stream starts warm.
            junk = persist.tile([P, P], f32, tag="junk", name="junk")
            nc.gpsimd.memset(junk[:], 0)
            jps = ps.tile([P, P], f32, tag="ps7", name="jps")
            for _ in range(32):
                nc.tensor.matmul(jps[:], junk[:], junk[:], start=True, stop=True)

            # ---- stage 1: T(E|O)^T[j', m] = sum_i' x(a|b)[i',j'] c0tp[i',m]
            for h in range(2):
                src = xa_d if h == 0 else xb_d
                for g in range(4):  # j'-column groups of 512
                    bk = banks()
                    for kq in range(HT // KQ):
                        if h == 0 and g == 0 and kq == 0:
                            # fine-grained first chunk: first matmuls can
                            # start after ~512 KB instead of 2 MB
                            for ko in range(KQ):
                                nc.scalar.dma_start(
                                    c0tp_sb[:, ko, :],
                                    c0tp_d[ko * P:(ko + 1) * P, :],
                                )
                                if ko == 0:
                                    xt = xin.tile(
                                        [P, KQ, G], f32r, tag="xt", name="xt"
                                    )
                                nc.sync.dma_start(
                                    xt[:, ko, :],
                                    src[ko * P:(ko + 1) * P, 0:G],
                                )
                        else:
                            if h == 0 and g == 0:
                                nc.scalar.dma_start(
                                    c0tp_sb[:, kq * KQ:(kq + 1) * KQ, :],
                                    c0tp_d[
                                        kq * KQ * P:(kq + 1) * KQ * P, :
                                    ].rearrange("(o p) m -> p o m", p=P),
                                )
                            xt = xin.tile([P, KQ, G], f32r, tag="xt", name="xt")
                            nc.sync.dma_start(
                                xt[:],
                                src[
                                    kq * KQ * P:(kq + 1) * KQ * P,
                                    g * G:(g + 1) * G,
                                ].rearrange("(o p) n -> p o n", p=P),
                            )
                        for ko in range(KQ):
                            it = kq * KQ + ko
                            for jb in range(4):
                                nc.tensor.matmul(
                                    bk[jb][:],
                                    xt[:, ko, jb * P:(jb + 1) * P],
                                    c0tp_sb[:, it, :],
                                    start=(it == 0),
                                    stop=(it == HT - 1),
                                )
                        if h == 0 and g < 2 and (g > 0 or kq >= 2):
                            # idle-bank warm fillers across the early
                            # HBM-starved chunk boundaries
                            ftag = "ps4" if g == 0 else "ps0"
                            fps = ps.tile(
                                [P, P], f32, tag=ftag, name="fps"
                            )
                            for _ in range(2):
                                nc.tensor.matmul(
                                    fps[:], junk[:], junk[:],
                                    start=True, stop=True,
                                )
                    for jb in range(4):
                        nc.vector.tensor_copy(
                            t_sb[h][:, g * 4 + jb, :], bk[jb][:]
                        )
                if h == 0:
                    # level-2 butterfly on TE' (runs on DVE under the
                    # TO-half matmuls): t2o = lo - hi; t0[lo] += hi
                    for jt in range(QT):
                        nc.vector.tensor_tensor(
                            t2o_sb[:, jt, :],
                            t_sb[0][:, jt, :],
                            t_sb[0][:, QT + jt, :],
                            mybir.AluOpType.subtract,
                        )
                        nc.vector.tensor_tensor(
                            t_sb[0][:, jt, :],
                            t_sb[0][:, jt, :],
                            t_sb[0][:, QT + jt, :],
                            mybir.AluOpType.add,
                        )

            # ---- stage 2 ----
            # v%4==0 and v%4==2 sections: 1024-deep contraction
            for sec in range(2):  # 0: t2e (=t0[:QT]), 1: t2o
                lhs = t_sb[0] if sec == 0 else t2o_sb
                for blk in range(2):  # 512 output columns each
                    bk = banks()
                    for jq in range(QT // KQ):
                        ct = cin.tile([P, KQ, G], f32r, tag="ct", name="ct")
                        nc.sync.dma_start(
                            ct[:],
                            c1v02_d[
                                jq * KQ * P:(jq + 1) * KQ * P,
                                (2 * sec + blk) * G:(2 * sec + blk + 1) * G,
                            ].rearrange("(o p) v -> p o v", p=P),
                        )
                        for jo in range(KQ):
                            jt = jq * KQ + jo
                            for mb in range(4):
                                nc.tensor.matmul(
                                    bk[mb][:],
                                    lhs[:, jt, mb * P:(mb + 1) * P],
                                    ct[:, jo, :],
                                    start=(jt == 0),
                                    stop=(jt == QT - 1),
                                )
                    for mb in range(4):
                        ot = osb.tile([P, G], f32, tag="ot", name="ot")
                        drain(bk[mb], mb, ot[:])
                        nc.gpsimd.dma_start(
                            out_d[
                                mb * P:(mb + 1) * P,
                                (2 * sec + blk) * G:(2 * sec + blk + 1) * G,
                            ],
                            ot[:],
                        )
            # v odd section: 2048-deep contraction over TO
            for vg in range(4):  # 512 output columns each
                bk = banks()
                for jq in range(HT // KQ):
                    ct = cin.tile([P, KQ, G], f32r, tag="ct", name="ct")
                    nc.sync.dma_start(
                        ct[:],
                        c1vo_d[
                            jq * KQ * P:(jq + 1) * KQ * P,
                            vg * G:(vg + 1) * G,
                        ].rearrange("(o p) v -> p o v", p=P),
                    )
                    for jo in range(KQ):
                        jt = jq * KQ + jo
                        for mb in range(4):
                            nc.tensor.matmul(
                                bk[mb][:],
                                t_sb[1][:, jt, mb * P:(mb + 1) * P],
                                ct[:, jo, :],
                                start=(jt == 0),
                                stop=(jt == HT - 1),
                            )
                for mb in range(4):
                    ot = osb.tile([P, G], f32, tag="ot", name="ot")
                    drain(bk[mb], mb, ot[:])
                    if vg == 3:
                        eng = nc.sync if mb % 2 == 0 else nc.scalar
                    else:
                        eng = nc.gpsimd
                    eng.dma_start(
                        out_d[
                            mb * P:(mb + 1) * P,
                            2048 + vg * G:2048 + (vg + 1) * G,
                        ],
                        ot[:],
                    )
    nc.compile()
    return nc


def _get_nc():
    if "nc" not in _CACHE:
        _CACHE["nc"] = _build()
    return _CACHE["nc"]


def _dct_basis_t():
    """C^T as float32 [N, N]: C^T[i, k] = cos(pi*(2i+1)*k/(2N)).

    Matches the reference's float32 jnp computation (fp32 argument
    arithmetic) so basis rounding does not diverge from the oracle."""
    if "ct" in _CACHE:
        return _CACHE["ct"]
    ct = None
    try:
        import jax
        import jax.numpy as jnp

        cpus = jax.devices("cpu")
        with jax.default_device(cpus[0]):
            k = jnp.arange(N, dtype=jnp.float32)[:, None]
            i = jnp.arange(N, dtype=jnp.float32)[None, :]
            c = jnp.cos((jnp.pi / (2.0 * N)) * (2.0 * i + 1.0) * k)
            ct = np.ascontiguousarray(np.asarray(c).T)
    except Exception:
        pass
    if ct is None:
        k = np.arange(N, dtype=np.float32)[:, None]
        i = np.arange(N, dtype=np.float32)[None, :]
        s = math.pi / (2.0 * N)
        arg = (s * (2.0 * i + 1.0)).astype(np.float32) * k
        ct = np.ascontiguousarray(np.cos(arg.astype(np.float32)).T)
    _CACHE["ct"] = ct
    return ct


def _in_maps(x):
    x = np.asarray(x, dtype=np.float32)
    ct = _dct_basis_t()

    # level-1 host folds (exact up to fp32 rounding)
    xE = x[:H] + x[:H - 1:-1]
    xO = x[:H] - x[:H - 1:-1]
    quads = {}
    for tag, xf in (("E", xE), ("O", xO)):
        xa = xf[:, :H] + xf[:, :H - 1:-1]
        # permute xa columns [0..Q-1, H-1..Q] so stage-1 tiles align
        # level-2 reflection partners at equal partition offsets
        quads[tag + "a"] = np.ascontiguousarray(
            np.concatenate([xa[:, :Q], xa[:, :Q - 1:-1]], axis=1)
        )
        quads[tag + "b"] = np.ascontiguousarray(xf[:, :H] - xf[:, :H - 1:-1])

    # stage-2 bases
    c1v02 = np.empty((Q, H), dtype=np.float32)
    c1v02[:, :Q] = ct[:Q, 0::4]  # C[4v'', j''], j'' rows
    c1v02[:, Q:] = ct[:Q, 2::4]
    c1vo = np.ascontiguousarray(ct[:H, 1::2])

    maps = []
    for c in range(NCORES):
        par = 0 if c < 4 else 1
        base = 1024 * (c % 4)
        maps.append(
            {
                "xa": quads[("E" if par == 0 else "O") + "a"],
                "xb": quads[("E" if par == 0 else "O") + "b"],
                "c0tp": np.ascontiguousarray(
                    ct[:H, base + par:base + 1024 + par:2]
                ),
                "c1v02": c1v02,
                "c1vo": c1vo,
            }
        )
    return maps


def _assemble(results):
    full = np.empty((N, N), dtype=np.float32)
    for c in range(NCORES):
        par = 0 if c < 4 else 1
        base = 1024 * (c % 4)
        rows = full[base + par:base + 1024 + par:2]
        dev = results[c]["out"]
        rows[:, 0::4] = dev[:, 0:1024]
        rows[:, 2::4] = dev[:, 1024:2048]
        rows[:, 1::2] = dev[:, 2048:4096]
    return full


def _run(x, **kwargs):
    nc = _get_nc()
    res = run_bass_kernel_spmd(
        nc, _in_maps(x), core_ids=list(range(NCORES)), **kwargs
    )
    return _assemble(res.results), res


def kernel(x):
    out, _ = _run(x)
    return out


# revision 10
# speedup vs baseline: 1.0238x; 1.0238x over previous
"""2D DCT-II (4096x4096, fp32) on 8 TRN2 NeuronCores.

out = C0 @ x @ C1^T with C0 = C1 = C, C[k, i] = cos(pi*(2i+1)*k/(2N)).

Fast-DCT folding via the basis reflection symmetries
    C[u, N-1-i]   = (-1)^u     * C[u, i]        (level 1, both axes)
    C[v, N/2-1-j] = (-1)^(v/2) * C[v, j]  (v even; level 2, column axis)

level 1 (both stages, folded on the HOST -> half FLOPs + half HBM):
  - cores 0-3 own even output rows u, cores 4-7 odd rows;
  - host supplies doubly-folded x quarters xa/xb [2048,2048] (feeding
    even-v / odd-v outputs) and basis slices;
level 2 (column axis only, even v split into v%4==0 / v%4==2):
  - xa arrives with its columns permuted [0..1023, 2047..1024] so the
    stage-1 intermediate tiles pair reflection partners at identical
    partition offsets; a 16-op DVE butterfly (running under stage 1's
    remaining matmuls) then yields the quarter-folded T2E/T2O, and the
    v%4 sections contract over only 1024 elements.

Device pipeline per core (all matmuls fp32r = full-rate FP22):
  stage 1: T(E|O)^T[j', m] = sum_i' x(a|b)[i', j'] * c0tp[i', m]
     lhsT = x tile (streamed, 1 MB DMAs), rhs = c0tp (SBUF-resident)
     -> 512 matmuls; intermediates land transposed in SBUF, exactly the
     stationary layout stage 2 needs.
  butterfly: t2e (in-place over TE') / t2o = TE'[j''] -+ TE'[j''+1024]
  stage 2: v%4==0: sum_{j''<1024} t2e^T * C[4v'', j'']     ( 64 matmuls)
           v%4==2: sum_{j''<1024} t2o^T * C[4v''+2, j'']   ( 64 matmuls)
           v odd:  sum_{j'<2048}  TO^T  * C[2v'+1, j']     (256 matmuls)
     rhs = basis (streamed), lhsT = intermediates (SBUF-resident).
  Output leaves in section-packed columns [v0 | v2 | vodd]; the host
  de-interleaves (pure numpy slicing).

PSUM: 4-bank accumulation groups alternate between two bank sets so a
group's drain (DVE/ACT copies, alternating) overlaps the next group's
matmuls. Total per-core: 896 matmuls (~203 us PE) + ~60 MB HBM.
"""

import math

import numpy as np

import concourse.mybir as mybir
import concourse.tile as tile
from concourse import bacc
from concourse.bass_utils import run_bass_kernel_spmd

N = 4096
H = N // 2  # 2048: level-1 folded contraction
Q = N // 4  # 1024: level-2 folded contraction
P = 128
HT = H // P  # 16
QT = Q // P  # 8
NCORES = 8
RB = 512  # output rows per core
G = 512  # column-group / matmul moving width
KQ = 4  # k-tiles per streaming DMA (1 MB)

f32 = mybir.dt.float32
f32r = mybir.dt.float32r

_CACHE = {}


def _build():
    nc = bacc.Bacc("TRN2", target_bir_lowering=False, debug=False)
    xa_d = nc.dram_tensor("xa", [H, H], f32r, kind="ExternalInput")
    xb_d = nc.dram_tensor("xb", [H, H], f32r, kind="ExternalInput")
    c0tp_d = nc.dram_tensor("c0tp", [H, RB], f32r, kind="ExternalInput")
    c1v02_d = nc.dram_tensor("c1v02", [Q, H], f32r, kind="ExternalInput")
    c1vo_d = nc.dram_tensor("c1vo", [H, H], f32r, kind="ExternalInput")
    out_d = nc.dram_tensor("out", [RB, N], f32, kind="ExternalOutput")

    state = {"ggc": 0}

    with tile.TileContext(nc) as tc:
        with (
            tc.tile_pool(name="persist", bufs=1) as persist,
            tc.tile_pool(name="xin", bufs=4) as xin,
            tc.tile_pool(name="cin", bufs=5) as cin,
            tc.tile_pool(name="osb", bufs=3) as osb,
            tc.tile_pool(name="ps", bufs=1, space="PSUM") as ps,
        ):
            c0tp_sb = persist.tile([P, HT, RB], f32r, tag="c0", name="c0tp_sb")
            # TE' (permuted) / TO intermediates: [j', m] as [128, 16, 512]
            t_sb = [
                persist.tile([P, HT, RB], f32r, tag=f"t{h}", name=f"t{h}_sb")
                for h in range(2)
            ]
            # level-2 odd-sign butterfly output (t2e overwrites t0 in place)
            t2o_sb = persist.tile([P, QT, RB], f32r, tag="t2o", name="t2o_sb")

            def banks(n=4):
                g = state["ggc"]
                state["ggc"] += 1
                return [
                    ps.tile(
                        [P, G], f32, tag=f"ps{(g % 2) * 4 + i}",
                        name=f"ps{(g % 2) * 4 + i}",
                    )
                    for i in range(n)
                ]

            def drain(bk, mb, dst):
                # alternate DVE/ACT so section-end drains parallelize
                if mb % 2 == 0:
                    nc.vector.tensor_copy(dst, bk[:])
                else:
                    nc.scalar.copy(dst, bk[:])

            # PE warm-up: the HAM clock gate needs ~3.4 us of sustained
            # matmul activity to lift the PE from 1.2 to 2.4 GHz, and the
            # first real matmul can't start until ~0.5 MB of operands
            # land (~10 us incl. preamble). Chew zeros meanwhile so the
            # real stream starts warm.
            junk = persist.tile([P, P], f32, tag="junk", name="junk")
            nc.gpsimd.memset(junk[:], 0)
            jps = ps.tile([P, P], f32, tag="ps7", name="jps")
            for _ in range(32):
                nc.tensor.matmul(jps[:], junk[:], junk[:], start=True, stop=True)

            # ---- stage 1: T(E|O)^T[j', m] = sum_i' x(a|b)[i',j'] c0tp[i',m]
            for h in range(2):
                src = xa_d if h == 0 else xb_d
                for g in range(4):  # j'-column groups of 512
                    bk = banks()
                    for kq in range(HT // KQ):
                        if h == 0 and g == 0 and kq == 0:
                            # fine-grained first chunk: first matmuls can
                            # start after ~512 KB instead of 2 MB
                            for ko in range(KQ):
                                nc.scalar.dma_start(
                                    c0tp_sb[:, ko, :],
                                    c0tp_d[ko * P:(ko + 1) * P, :],
                                )
                                if ko == 0:
                                    xt = xin.tile(
                                        [P, KQ, G], f32r, tag="xt", name="xt"
                                    )
                                nc.sync.dma_start(
                                    xt[:, ko, :],
                                    src[ko * P:(ko + 1) * P, 0:G],
                                )
                        else:
                            if h == 0 and g == 0:
                                nc.scalar.dma_start(
                                    c0tp_sb[:, kq * KQ:(kq + 1) * KQ, :],
                                    c0tp_d[
                                        kq * KQ * P:(kq + 1) * KQ * P, :
                                    ].rearrange("(o p) m -> p o m", p=P),
                                )
                            xt = xin.tile([P, KQ, G], f32r, tag="xt", name="xt")
                            nc.sync.dma_start(
                                xt[:],
                                src[
                                    kq * KQ * P:(kq + 1) * KQ * P,
                                    g * G:(g + 1) * G,
                                ].rearrange("(o p) n -> p o n", p=P),
                            )
                        for ko in range(KQ):
                            it = kq * KQ + ko
                            for jb in range(4):
                                nc.tensor.matmul(
                                    bk[jb][:],
                                    xt[:, ko, jb * P:(jb + 1) * P],
                                    c0tp_sb[:, it, :],
                                    start=(it == 0),
                                    stop=(it == HT - 1),
                                )
                        if h == 0 and g < 2 and (g > 0 or kq >= 2):
                            # idle-bank warm fillers across the early
                            # HBM-starved chunk boundaries
                            ftag = "ps4" if g == 0 else "ps0"
                            fps = ps.tile(
                                [P, P], f32, tag=ftag, name="fps"
                            )
                            for _ in range(2):
                                nc.tensor.matmul(
                                    fps[:], junk[:], junk[:],
                                    start=True, stop=True,
                                )
                    for jb in range(4):
                        nc.vector.tensor_copy(
                            t_sb[h][:, g * 4 + jb, :], bk[jb][:]
                        )
                if h == 0:
                    # level-2 butterfly on TE' (runs on DVE under the
                    # TO-half matmuls): t2o = lo - hi; t0[lo] += hi
                    for jt in range(QT):
                        nc.vector.tensor_tensor(
                            t2o_sb[:, jt, :],
                            t_sb[0][:, jt, :],
                            t_sb[0][:, QT + jt, :],
                            mybir.AluOpType.subtract,
                        )
                        nc.vector.tensor_tensor(
                            t_sb[0][:, jt, :],
                            t_sb[0][:, jt, :],
                            t_sb[0][:, QT + jt, :],
                            mybir.AluOpType.add,
                        )

            # ---- stage 2 ----
            # v%4==0 and v%4==2 sections: 1024-deep contraction
            for sec in range(2):  # 0: t2e (=t0[:QT]), 1: t2o
                lhs = t_sb[0] if sec == 0 else t2o_sb
                for blk in range(2):  # 512 output columns each
                    bk = banks()
                    for jq in range(QT // KQ):
                        ct = cin.tile([P, KQ, G], f32r, tag="ct", name="ct")
                        nc.sync.dma_start(
                            ct[:],
                            c1v02_d[
                                jq * KQ * P:(jq + 1) * KQ * P,
                                (2 * sec + blk) * G:(2 * sec + blk + 1) * G,
                            ].rearrange("(o p) v -> p o v", p=P),
                        )
                        for jo in range(KQ):
                            jt = jq * KQ + jo
                            for mb in range(4):
                                nc.tensor.matmul(
                                    bk[mb][:],
                                    lhs[:, jt, mb * P:(mb + 1) * P],
                                    ct[:, jo, :],
                                    start=(jt == 0),
                                    stop=(jt == QT - 1),
                                )
                    for mb in range(4):
                        ot = osb.tile([P, G], f32, tag="ot", name="ot")
                        drain(bk[mb], mb, ot[:])
                        nc.gpsimd.dma_start(
                            out_d[
                                mb * P:(mb + 1) * P,
                                (2 * sec + blk) * G:(2 * sec + blk + 1) * G,
                            ],
                            ot[:],
                        )
            # v odd section: 2048-deep contraction over TO
            for vg in range(4):  # 512 output columns each
                bk = banks()
                for jq in range(HT // KQ):
                    ct = cin.tile([P, KQ, G], f32r, tag="ct", name="ct")
                    nc.sync.dma_start(
                        ct[:],
                        c1vo_d[
                            jq * KQ * P:(jq + 1) * KQ * P,
                            vg * G:(vg + 1) * G,
                        ].rearrange("(o p) v -> p o v", p=P),
                    )
                    for jo in range(KQ):
                        jt = jq * KQ + jo
                        for mb in range(4):
                            nc.tensor.matmul(
                                bk[mb][:],
                                t_sb[1][:, jt, mb * P:(mb + 1) * P],
                                ct[:, jo, :],
                                start=(jt == 0),
                                stop=(jt == HT - 1),
                            )
                for mb in range(4):
                    ot = osb.tile([P, G], f32, tag="ot", name="ot")
                    drain(bk[mb], mb, ot[:])
                    if vg == 3:
                        eng = nc.sync if mb % 2 == 0 else nc.scalar
                    else:
                        eng = nc.gpsimd
                    eng.dma_start(
                        out_d[
                            mb * P:(mb + 1) * P,
                            2048 + vg * G:2048 + (vg + 1) * G,
                        ],
                        ot[:],
                    )
    nc.compile()
    return nc


def _get_nc():
    if "nc" not in _CACHE:
        _CACHE["nc"] = _build()
    return _CACHE["nc"]


def _dct_basis_t():
    """C^T as float32 [N, N]: C^T[i, k] = cos(pi*(2i+1)*k/(2N)).

    Matches the reference's float32 jnp computation (fp32 argument
    arithmetic) so basis rounding does not diverge from the oracle."""
    if "ct" in _CACHE:
        return _CACHE["ct"]
    ct = None
    try:
        import jax
        import jax.numpy as jnp

        cpus = jax.devices("cpu")
        with jax.default_device(cpus[0]):
            k = jnp.arange(N, dtype=jnp.float32)[:, None]
            i = jnp.arange(N, dtype=jnp.float32)[None, :]
            c = jnp.cos((jnp.pi / (2.0 * N)) * (2.0 * i + 1.0) * k)
            ct = np.ascontiguousarray(np.asarray(c).T)
    except Exception:
        pass
    if ct is None:
        k = np.arange(N, dtype=np.float32)[:, None]
        i = np.arange(N, dtype=np.float32)[None, :]
        s = math.pi / (2.0 * N)
        arg = (s * (2.0 * i + 1.0)).astype(np.float32) * k
        ct = np.ascontiguousarray(np.cos(arg.astype(np.float32)).T)
    _CACHE["ct"] = ct
    return ct


def _in_maps(x):
    x = np.asarray(x, dtype=np.float32)
    ct = _dct_basis_t()

    # level-1 host folds (exact up to fp32 rounding)
    xE = x[:H] + x[:H - 1:-1]
    xO = x[:H] - x[:H - 1:-1]
    quads = {}
    for tag, xf in (("E", xE), ("O", xO)):
        xa = xf[:, :H] + xf[:, :H - 1:-1]
        # permute xa columns [0..Q-1, H-1..Q] so stage-1 tiles align
        # level-2 reflection partners at equal partition offsets
        quads[tag + "a"] = np.ascontiguousarray(
            np.concatenate([xa[:, :Q], xa[:, :Q - 1:-1]], axis=1)
        )
        quads[tag + "b"] = np.ascontiguousarray(xf[:, :H] - xf[:, :H - 1:-1])

    # stage-2 bases
    c1v02 = np.empty((Q, H), dtype=np.float32)
    c1v02[:, :Q] = ct[:Q, 0::4]  # C[4v'', j''], j'' rows
    c1v02[:, Q:] = ct[:Q, 2::4]
    c1vo = np.ascontiguousarray(ct[:H, 1::2])

    maps = []
    for c in range(NCORES):
        par = 0 if c < 4 else 1
        base = 1024 * (c % 4)
        maps.append(
            {
                "xa": quads[("E" if par == 0 else "O") + "a"],
                "xb": quads[("E" if par == 0 else "O") + "b"],
                "c0tp": np.ascontiguousarray(
                    ct[:H, base + par:base + 1024 + par:2]
                ),
                "c1v02": c1v02,
                "c1vo": c1vo,
            }
        )
    return maps


def _assemble(results):
    full = np.empty((N, N), dtype=np.float32)
    for c in range(NCORES):
        par = 0 if c < 4 else 1
        base = 1024 * (c % 4)
        rows = full[base + par:base + 1024 + par:2]
        dev = results[c]["out"]
        rows[:, 0::4] = dev[:, 0:1024]
        rows[:, 2::4] = dev[:, 1024:2048]
        rows[:, 1::2] = dev[:, 2048:4096]
    return full


def _run(x, **kwargs):
    nc = _get_nc()
    in_maps = _in_maps(x)
    last = None
    for attempt in range(3):
        try:
            res = run_bass_kernel_spmd(
                nc, in_maps, core_ids=list(range(NCORES)), **kwargs
            )
            return _assemble(res.results), res
        except Exception as e:  # transient NRT/device faults happen rarely
            last = e
    raise last


def kernel(x):
    out, _ = _run(x)
    return out
